# revision 4
# baseline (speedup 1.0000x reference)
"""Trainium2 Bass kernel for nn_PlatonicConv (linear-attention GNN message passing).

Math (reference):
  q = rope(x@Wq + bq, phase);  k = rope(ones, phase);  v = x@Wv + bv
  phase[n, g, p] = pos[n, :] . freqs[g, 0, p, :]
  KV_b[g] = (1/AVG) * sum_{n in graph b} k[n,g,:] (x) v[n,g,:]
  out[n]  = concat_g( q'[n,g,:] @ KV_b[g] ) @ Wo + bo

Device formulation (per core, 8 graphs/core, data-parallel):
  Stage 1 (per 512-node chunk): q = x @ WqA (A-ordered cols, fp8 DoubleRow)
    into 3 PSUM banks -> cast to feature-major SBUF planes (qE/qO full
    [128,NP] planes for rope pairs t<128; packed [128,NP/2] half planes for
    t>=128).  Per-graph XK/KV/M_b work is interleaved between chunks.
  Rope: wide (2560-col) vector TENSOR_TENSORs on the planes (2x DVE mode),
    trig cos/sin DMA'd from host.  q2 half-planes re-stacked to a [128,NP]
    tile via SBUF->SBUF DMA.
  Per graph g: XK[c,kcol] = sum_n x[n,c]*k[n,kcol] (fp8 DoubleRow over
    node-tile pairs; node-major x and A-col-ordered k packed in one stream);
    KV^T = Wv^T @ XK (fp8 DoubleRow over feature-chunk pairs);
    arena = KV^T * blockdiag-mask (one vector TT, evacuates PSUM);
    M_b = arena-blocks @ Wo-chunks (6 small PE matmuls, bf16);
    scale folding: Wq x8, Wv x2 (fp8-friendly ranges) -> host divides by 16.
  Out phase: out[:, g-slot] = sum_chunks M_b-chunk^T @ q'-chunk, all 8 PSUM
    banks rotating across (graph, window, jchunk); bf16 output; host adds bo.

Self-contained: hardcodes shapes; shards/pads on host inside kernel().
"""

import math
import os
from contextlib import ExitStack

import ml_dtypes
import numpy as np

import concourse.bacc as bacc_mod
import concourse.bass as bass
import concourse.mybir as mybir
import concourse.tile as tile
from concourse.bass_utils import run_bass_kernel_spmd


def _ensure_ntff_hook():
    """Register the axon NTFF profile hook if the image's antenv lacks it."""
    try:
        import antenv.axon_hooks  # noqa: F401

        return True
    except ImportError:
        pass
    try:
        import sys
        import types

        import antenv
        from trn_agent_boot.trn_boot import _ntff_profile_via_ctypes

        mod = types.ModuleType("antenv.axon_hooks")
        _hook = [None]
        mod.set_axon_ntff_profile_hook = lambda h: _hook.__setitem__(0, h)
        mod.get_axon_ntff_profile_hook = lambda: _hook[0]
        sys.modules["antenv.axon_hooks"] = mod
        antenv.axon_hooks = mod
        mod.set_axon_ntff_profile_hook(
            _ntff_profile_via_ctypes("/opt/axon/libaxon_pjrt.so")
        )
        return True
    except Exception:
        return False


FP32 = mybir.dt.float32
BF16 = mybir.dt.bfloat16
FP8 = mybir.dt.float8e4
AF = mybir.ActivationFunctionType
DR = mybir.MatmulPerfMode.DoubleRow
DR_SITES = os.environ.get("PLATCONV_DR", "")

N = 32768
C = 384
E = 384
G = 12
D = 32
P = 16
SD = 3
NUM_GRAPHS = 64
NCORES = 8
GPD = NUM_GRAPHS // NCORES  # graphs per device
AVG = float(N) / NUM_GRAPHS  # 512.0
NT = 192  # rope pairs = G*P
W = 512  # chunk width (PSUM bank)

QSC = 1.0  # Wq prescale (unused at bf16)
VSC = 1.0  # Wv prescale (unused at bf16)


def _a_order_cols():
    """perm such that A-order column r is original q-dim perm[r]."""
    perm = np.empty(E, dtype=np.int64)
    for r in range(E):
        if r < 128:
            t, odd = r, 0
        elif r < 256:
            t, odd = r - 128, 1
        elif r < 320:
            t, odd = 128 + (r - 256), 0
        else:
            t, odd = 128 + (r - 320), 1
        perm[r] = (t // 16) * 32 + 2 * (t % 16) + odd
    return perm


_APERM = _a_order_cols()

_CACHE = {}


def _build(slot: int, has_bias: bool):
    key = (slot, has_bias)
    if key in _CACHE:
        return _CACHE[key]

    NP = GPD * slot
    TPS = -(-slot // 128)  # kx tiles per graph (node tiles, zero-padded)
    KXT = GPD * TPS
    NCH = NP // W
    HALF = NP // 2
    TOPCH = -(-NCH // 2)  # chunks packed into partition rows 0:64
    TOPC = TOPCH * W
    assert NP % W == 0 and slot % 64 == 0

    nc = bacc_mod.Bacc()

    xt_d = nc.declare_dram_parameter("xt", [NCH, 128, 3 * W], BF16, isOutput=False)
    kx_d = nc.declare_dram_parameter("kx", [128, KXT * 768], BF16, isOutput=False)
    trig_d = nc.declare_dram_parameter("trig", [128, 2 * NP + 2 * TOPC], BF16, isOutput=False)
    wp_d = nc.declare_dram_parameter("wp", [128, 5 * 384], BF16, isOutput=False)
    w8_d = nc.declare_dram_parameter("w8", [128, 6 * 384], BF16, isOutput=False)
    if has_bias:
        xb_d = nc.declare_dram_parameter("xb", [NCH, 1, W], BF16, isOutput=False)
        wb_d = nc.declare_dram_parameter("wb", [2, E], BF16, isOutput=False)
    out_d = nc.declare_dram_parameter("outt", [128, 3 * NP], BF16, isOutput=True)

    with ExitStack() as ctx:
        tc = ctx.enter_context(tile.TileContext(nc))

        consts = ctx.enter_context(tc.tile_pool(name="consts", bufs=1))
        xtp = ctx.enter_context(tc.tile_pool(name="xtp", bufs=3))
        gstr = ctx.enter_context(tc.tile_pool(name="gstr", bufs=4))
        planes = ctx.enter_context(tc.tile_pool(name="planes", bufs=1))
        xkp = ctx.enter_context(tc.tile_pool(name="xkp", bufs=2))
        aren = ctx.enter_context(tc.tile_pool(name="aren", bufs=2))
        mbp = ctx.enter_context(tc.tile_pool(name="mbp", bufs=8))
        outp = ctx.enter_context(tc.tile_pool(name="outp", bufs=2))
        psum = ctx.enter_context(tc.tile_pool(name="psum", bufs=1, space="PSUM"))

        def pbank(tag):
            return psum.tile([128, W], FP32, tag=tag, name=tag)

        # ---- weights (sync ring): wq first (gates chunk 0), wv after c0 ----
        w8 = consts.tile([128, 6 * 384], BF16, tag="w8", name="w8")
        nc.sync.dma_start(w8[:, 0 : 3 * 384], w8_d[:, 0 : 3 * 384])
        wp = consts.tile([128, 5 * 384], BF16, tag="wp", name="wp")
        wos_t = [wp[:, i * 384 : (i + 1) * 384] for i in range(3)]
        mask_t = wp[:, 4 * 384 : 5 * 384]
        wq8 = w8[:, 0 : 3 * 384].rearrange("p (s c) -> p s c", c=384)
        wv8 = w8[:, 3 * 384 : 6 * 384].rearrange("p (s c) -> p s c", c=384)
        if has_bias:
            wb = consts.tile([2, E], BF16, tag="wb", name="wb")
            nc.sync.dma_start(wb[:], wb_d[:])

        # ---- persistent SBUF planes ----
        qE = planes.tile([128, NP], BF16, tag="qE", name="qE")
        qO = planes.tile([128, NP], BF16, tag="qO", name="qO")
        m1 = planes.tile([128, NP], BF16, tag="m1", name="m1")
        m2 = planes.tile([128, NP], BF16, tag="m2", name="m2")
        tf = planes.tile([128, NP], BF16, tag="tf", name="tf")
        hE = planes.tile([128, TOPC], BF16, tag="hE", name="hE")
        hO = planes.tile([128, TOPC], BF16, tag="hO", name="hO")
        n1 = planes.tile([128, TOPC], BF16, tag="n1", name="n1")
        n2 = planes.tile([128, TOPC], BF16, tag="n2", name="n2")
        th = planes.tile([128, TOPC], BF16, tag="th", name="th")
        q2 = planes.tile([128, NP], BF16, tag="q2", name="q2")
        trig = planes.tile([128, 2 * NP + 2 * TOPC], BF16, tag="trig", name="trig")
        clf = trig[:, 0:NP]
        clh = trig[:, 2 * NP : 2 * NP + TOPC]
        slh = trig[:, 2 * NP + TOPC : 2 * NP + 2 * TOPC]

        # per-graph k/x streaming + trig loads on the gpsimd SWDGE ring
        kx_g = []

        def load_graph_inputs(j):
            t = gstr.tile([128, TPS * 768], BF16, tag="kx", name="kx")
            nc.gpsimd.dma_start(t[:], kx_d[:, j * TPS * 768 : (j + 1) * TPS * 768])
            return t

        kx_g.append(load_graph_inputs(0))
        # clh/slh top-partition halves (needed by rope_half_piece(0) early)
        nc.gpsimd.dma_start(
            trig[0:64, 2 * NP : 2 * NP + 2 * TOPC],
            trig_d[0:64, 2 * NP : 2 * NP + 2 * TOPC],
        )
        kx_g.append(load_graph_inputs(1))
        nc.gpsimd.dma_start(trig[:, 0:HALF], trig_d[:, 0:HALF])
        nc.gpsimd.dma_start(trig[:, NP : NP + HALF], trig_d[:, NP : NP + HALF])
        kx_g.append(load_graph_inputs(2))
        kx_g.append(load_graph_inputs(3))
        nc.gpsimd.dma_start(trig[:, HALF:NP], trig_d[:, HALF:NP])
        nc.gpsimd.dma_start(trig[:, NP + HALF : 2 * NP], trig_d[:, NP + HALF : 2 * NP])
        nc.gpsimd.dma_start(
            trig[64:128, 2 * NP : 2 * NP + 2 * TOPC],
            trig_d[64:128, 2 * NP : 2 * NP + 2 * TOPC],
        )
        for j in range(4, GPD):
            kx_g.append(load_graph_inputs(j))

        # ---- per-graph XK / KV / M_b (emitted interleaved with chunks) ----
        mb_all = []

        def graph_stage(j):
            kxt = kx_g[j].rearrange("p (t s) -> p t s", s=768)
            xkb = [pbank("T3"), pbank("T4"), pbank("T5")]
            npair = TPS // 2
            for ci in range(3):
                xs = slice(E + 128 * ci, E + 128 * (ci + 1))
                if "X" in DR_SITES:
                    for tp in range(npair):
                        nc.tensor.matmul(
                            xkb[ci][:, 0:E],
                            kxt[:, 2 * tp : 2 * tp + 2, xs],
                            kxt[:, 2 * tp : 2 * tp + 2, 0:E],
                            start=(tp == 0),
                            stop=(tp == npair - 1 and TPS % 2 == 0),
                            perf_mode=DR,
                        )
                else:
                    for t in range(2 * npair):
                        nc.tensor.matmul(
                            xkb[ci][:, 0:E],
                            kxt[:, t, xs],
                            kxt[:, t, 0:E],
                            start=(t == 0),
                            stop=False,
                        )
                if TPS % 2 == 1:
                    nc.tensor.matmul(
                        xkb[ci][:, 0:E],
                        kxt[:, TPS - 1, xs],
                        kxt[:, TPS - 1, 0:E],
                        start=False,
                        stop=True,
                    )
                else:
                    assert "X" in DR_SITES
            xk = xkp.tile([128, 3 * E], BF16, tag="xk", name="xk")
            for ci in range(3):
                nc.scalar.activation(xk[:, ci * E : (ci + 1) * E], xkb[ci][:, 0:E], AF.Copy)
            xk3 = xk.rearrange("p (s c) -> p s c", c=384)

            kvb = pbank("T6")
            for cc in range(3):
                co = slice(128 * cc, 128 * (cc + 1))
                if "K" in DR_SITES:
                    nc.tensor.matmul(
                        kvb[:, co],
                        wv8[:, 0:2, co],
                        xk3[:, 0:2, co],
                        start=True,
                        stop=False,
                        perf_mode=DR,
                    )
                else:
                    for ci in range(2):
                        nc.tensor.matmul(
                            kvb[:, co],
                            wv8[:, ci, co],
                            xk3[:, ci, co],
                            start=(ci == 0),
                            stop=False,
                        )
                nc.tensor.matmul(
                    kvb[:, co],
                    wv8[:, 2, co],
                    xk3[:, 2, co],
                    start=False,
                    stop=True,
                )
            arena = aren.tile([128, 3 * 128], BF16, tag="arena", name="arena")
            nc.vector.tensor_mul(arena[:], kvb[:, 0 : 3 * 128], mask_t)

            # M_b chunks (A-order rows), bank T7 reused with cast between
            mb_sb = []
            mbsrc = [
                ((arena[:, 0:64], wos_t[0]), (arena[:, 128:192], wos_t[1])),
                ((arena[:, 64:128], wos_t[0]), (arena[:, 192:256], wos_t[1])),
                ((arena[:, 256:320], wos_t[2]), (arena[:, 320:384], wos_t[2])),
            ]
            for mc in range(3):
                mbb = pbank("T7")
                (a0, w0_), (a1, w1_) = mbsrc[mc]
                nc.tensor.matmul(mbb[0:64, 0:C], a0, w0_, start=True, stop=True)
                nc.tensor.matmul(mbb[64:128, 0:C], a1, w1_, start=True, stop=True)
                t = mbp.tile([128, C], BF16, tag=f"mb{mc}", name=f"mb{mc}")
                nc.scalar.activation(t[:], mbb[:, 0:C], AF.Copy)
                mb_sb.append(t)
            mb_all.append(mb_sb)

        # ------------------------------------------------------------------
        # Rope pieces: piece k covers full-plane cols [1024k, 1024k+1024)
        # (chunks 2k, 2k+1) and the matching packed half-plane region.
        # Emitted right after chunk 2k+1 so the vector engine ropes while
        # the scalar engine evacuates later chunks.
        # ------------------------------------------------------------------
        RPW = 2 * W

        def rope_piece(k, wlen=None):
            # full-plane cols [1024k, 1024k+wlen) = chunks 2k, 2k+1
            wlen = wlen if wlen is not None else RPW
            hs = slice(RPW * k, RPW * k + wlen)
            cs = slice(NP + RPW * k, NP + RPW * k + wlen)
            nc.vector.tensor_mul(m1[:, hs], qE[:, hs], clf[:, hs])
            nc.vector.tensor_mul(m2[:, hs], qO[:, hs], trig[:, cs])
            nc.vector.tensor_mul(tf[:, hs], qE[:, hs], trig[:, cs])
            nc.vector.tensor_mul(qE[:, hs], qO[:, hs], clf[:, hs])
            nc.vector.tensor_sub(qO[:, hs], m1[:, hs], m2[:, hs])
            nc.vector.tensor_add(m1[:, hs], tf[:, hs], qE[:, hs])

        def rope_half_piece(ch):
            # packed half-plane region for chunk ch + restack into q2
            h = 0 if ch < TOPCH else 1
            p = slice(64 * h, 64 * h + 64)
            c0 = (ch - TOPCH * h) * W
            hc = slice(c0, c0 + W)
            nc.vector.tensor_mul(n1[p, hc], hE[p, hc], clh[p, hc])
            nc.vector.tensor_mul(n2[p, hc], hO[p, hc], slh[p, hc])
            nc.vector.tensor_mul(th[p, hc], hE[p, hc], slh[p, hc])
            nc.vector.tensor_mul(hE[p, hc], hO[p, hc], clh[p, hc])
            nc.vector.tensor_sub(hO[p, hc], n1[p, hc], n2[p, hc])
            nc.vector.tensor_add(n1[p, hc], th[p, hc], hE[p, hc])
            ncol = slice(ch * W, (ch + 1) * W)
            nc.sync.dma_start(q2[0:64, ncol], hO[p, hc])
            nc.sync.dma_start(q2[64:128, ncol], n1[p, hc])

        # ------------------------------------------------------------------
        # Stage 1: Q projection per chunk (sync ring); graph-stage work for
        # graph ch-1 interleaved after chunk ch to pace the scheduler.
        # ------------------------------------------------------------------
        for ch in range(NCH):
            n0 = ch * W
            xtc = xtp.tile([128, 3 * W], BF16, tag="xtc", name="xtc")
            nc.sync.dma_start(xtc[:], xt_d[ch, :, :])
            if ch == 0:
                nc.sync.dma_start(w8[:, 3 * 384 : 6 * 384], w8_d[:, 3 * 384 : 6 * 384])
                nc.sync.dma_start(wp[:], wp_d[:])
            xtc3 = xtc.rearrange("p (s w) -> p s w", w=W)
            if has_bias:
                xbc = xtp.tile([1, W], BF16, tag="xbc", name="xbc")
                nc.sync.dma_start(xbc[:], xb_d[ch, :, :])

            bE = pbank("T0")
            bO = pbank("T1")
            b2 = pbank("T2")
            for ps, c0 in ((bE, 0), (bO, 128), (b2, 256)):
                cs = slice(c0, c0 + 128)
                if "Q" in DR_SITES:
                    nc.tensor.matmul(
                        ps[:],
                        wq8[:, 0:2, cs],
                        xtc3[:, 0:2, :],
                        start=True,
                        stop=False,
                        perf_mode=DR,
                    )
                else:
                    for ki in range(2):
                        nc.tensor.matmul(
                            ps[:],
                            wq8[:, ki, cs],
                            xtc3[:, ki, :],
                            start=(ki == 0),
                            stop=False,
                        )
                nc.tensor.matmul(
                    ps[:],
                    wq8[:, 2, cs],
                    xtc3[:, 2, :],
                    start=False,
                    stop=not has_bias,
                )
                if has_bias:
                    nc.tensor.matmul(
                        ps[:], wb[0:1, cs], xbc[:], start=False, stop=True
                    )
            # full planes: qE on scalar, qO on vector
            nc.scalar.activation(qE[:, n0 : n0 + W], bE[:], AF.Copy)
            nc.vector.tensor_copy(qO[:, n0 : n0 + W], bO[:])
            # half planes (packed): E2 rows 0:64, O2 rows 64:128 of b2
            if ch < TOPCH:
                hw = slice(ch * W, (ch + 1) * W)
                nc.scalar.activation(hE[0:64, hw], b2[0:64, :], AF.Copy)
                nc.vector.tensor_copy(hO[0:64, hw], b2[64:128, :])
            else:
                hw = slice((ch - TOPCH) * W, (ch - TOPCH + 1) * W)
                nc.vector.tensor_copy(hE[64:128, hw], b2[0:64, :])
                nc.scalar.activation(hO[64:128, hw], b2[64:128, :], AF.Copy)
            if 1 <= ch <= GPD:
                graph_stage(ch - 1)
            rope_half_piece(ch)
            if ch % 2 == 1:
                rope_piece(ch // 2)
            elif ch == NCH - 1:
                rope_piece(ch // 2, W)

        q0p, q1p = qO, m1

        # ------------------------------------------------------------------
        # Out phase: all 8 PSUM banks rotate across (graph, window, jchunk)
        # ------------------------------------------------------------------
        oti = 0
        for j in range(GPD):
            mb_sb = mb_all[j]
            slot0 = j * slot
            ost = outp.tile([128, 3 * slot], BF16, tag="ost", name="ost")
            for jc in range(3):
                js = slice(128 * jc, 128 * (jc + 1))
                o = 0
                while o < slot:
                    w = min(W, slot - o)
                    ob = pbank(f"T{oti % 8}")
                    nc.tensor.matmul(
                        ob[:, :w],
                        mb_sb[0][:, js],
                        q0p[:, slot0 + o : slot0 + o + w],
                        start=True,
                        stop=False,
                    )
                    nc.tensor.matmul(
                        ob[:, :w],
                        mb_sb[1][:, js],
                        q1p[:, slot0 + o : slot0 + o + w],
                        start=False,
                        stop=False,
                    )
                    nc.tensor.matmul(
                        ob[:, :w],
                        mb_sb[2][:, js],
                        q2[:, slot0 + o : slot0 + o + w],
                        start=False,
                        stop=True,
                    )
                    if oti % 2 == 0:
                        nc.scalar.activation(
                            ost[:, jc * slot + o : jc * slot + o + w],
                            ob[:, :w],
                            AF.Copy,
                        )
                    else:
                        nc.vector.tensor_copy(
                            ost[:, jc * slot + o : jc * slot + o + w], ob[:, :w]
                        )
                    oti += 1
                    o += w
                nc.sync.dma_start(
                    out_d[:, 3 * slot0 + jc * slot : 3 * slot0 + (jc + 1) * slot],
                    ost[:, jc * slot : (jc + 1) * slot],
                )

    nc.compile()

    _CACHE[key] = (nc, NP)
    return nc, NP


last_exec_time_ns = None
last_results = None


def kernel(x, pos, batch, Wq, bq, Wv, bv, Wo, bo, freqs):
    global last_exec_time_ns, last_results
    x = np.asarray(x, dtype=np.float32)
    pos = np.asarray(pos, dtype=np.float32)
    batch = np.asarray(batch).astype(np.int64)
    Wq = np.asarray(Wq, dtype=np.float32)
    bq = np.asarray(bq, dtype=np.float32)
    Wv = np.asarray(Wv, dtype=np.float32)
    bv = np.asarray(bv, dtype=np.float32)
    Wo = np.asarray(Wo, dtype=np.float32)
    bo = np.asarray(bo, dtype=np.float32)
    freqs = np.asarray(freqs, dtype=np.float32)

    counts = np.bincount(batch, minlength=NUM_GRAPHS)
    starts = np.concatenate([[0], np.cumsum(counts)])
    slot = max(512, int(math.ceil(counts.max() / 64.0)) * 64)
    # bv is folded nowhere in the XK formulation; reference setup zeroes it.
    assert not np.any(bv), "nonzero bv not supported"
    has_bias = bool(np.any(bq))

    nc, NP = _build(slot, has_bias)
    TPS = -(-slot // 128)
    KXT = GPD * TPS
    NCH = NP // W
    HALF = NP // 2
    TOPCH = -(-NCH // 2)
    TOPC = TOPCH * W
    bf = ml_dtypes.bfloat16
    def to8(a):
        return a.astype(bf)

    WqA = Wq[:, _APERM] * QSC
    wva = Wv * VSC
    wos = (Wo * (math.sqrt(2.0) / AVG)).astype(bf)

    # phase & trig on host (t = g*16+p, g-major)
    fr = freqs.reshape(NT, SD)
    phase = pos @ fr.T  # [N, 192] float32
    cphase = np.cos(phase)
    sphase = np.sin(phase)
    # k, A-col order: col 128cc+j = (c-s)*s2 of t=64cc+j; +64+j = (c+s)*s2
    s2 = 1.0 / math.sqrt(2.0)
    kA = np.empty((len(x), E), dtype=np.float32)
    for cc in range(3):
        tsl = slice(64 * cc, 64 * cc + 64)
        kA[:, 128 * cc : 128 * cc + 64] = (cphase[:, tsl] - sphase[:, tsl]) * s2
        kA[:, 128 * cc + 64 : 128 * cc + 128] = (cphase[:, tsl] + sphase[:, tsl]) * s2

    # block-diag mask [128, 384]
    mask = np.zeros((128, 3 * 128), dtype=bf)
    prow = np.arange(128) // 32
    jj = np.arange(128)
    gcol = np.where(jj < 64, jj // 16, (jj - 64) // 16)
    for cc in range(3):
        mask[:, 128 * cc : 128 * (cc + 1)] = (prow[:, None] == gcol[None, :]).astype(bf)

    # packed weights: bf16 [wos0..2 | pad | mask], fp8 [wq0..2 | wv0..2]
    wp = np.zeros((128, 5 * 384), dtype=bf)
    for i in range(3):
        wp[:, i * 384 : (i + 1) * 384] = wos[128 * i : 128 * (i + 1)]
    wp[:, 4 * 384 : 5 * 384] = mask
    w8 = np.zeros((128, 6 * 384), dtype=bf)
    for i in range(3):
        w8[:, (0 + i) * 384 : (1 + i) * 384] = to8(WqA[128 * i : 128 * (i + 1)])
        w8[:, (3 + i) * 384 : (4 + i) * 384] = to8(wva[128 * i : 128 * (i + 1)])

    in_maps = []
    for d in range(NCORES):
        xt_full = np.zeros((C, NP), dtype=np.float32)
        trig_c = np.zeros((NT, NP), dtype=bf)
        trig_s = np.zeros((NT, NP), dtype=bf)
        x_slot = np.zeros((NP, C), dtype=np.float32)
        k_slot = np.zeros((NP, E), dtype=np.float32)
        bias_col = np.zeros((NP,), dtype=bf)
        for lj in range(GPD):
            gb = d * GPD + lj
            s, e_, cnt = starts[gb], starts[gb + 1], counts[gb]
            if cnt == 0:
                continue
            o = lj * slot
            xt_full[:, o : o + cnt] = x[s:e_].T
            bias_col[o : o + cnt] = 1.0
            trig_c[:, o : o + cnt] = cphase[s:e_].T.astype(bf)
            trig_s[:, o : o + cnt] = sphase[s:e_].T.astype(bf)
            x_slot[o : o + cnt] = x[s:e_]
            k_slot[o : o + cnt] = kA[s:e_]
        # xt: [NCH, 128, 3*W], [ch, p, ki*W+u] = feature ki*128+p, node ch*W+u
        xt = to8(
            np.ascontiguousarray(
                xt_full.reshape(3, 128, NCH, W).transpose(2, 1, 0, 3)
            ).reshape(NCH, 128, 3 * W)
        )
        # kx: [128, KXT*768]: per-graph TPS-tile blocks: [0:384]=k, [384:768]=x
        kx3 = np.zeros((128, KXT, 768), dtype=np.float32)
        for lj in range(GPD):
            gb = d * GPD + lj
            s, e_, cnt = starts[gb], starts[gb + 1], counts[gb]
            kloc = np.zeros((TPS * 128, E), dtype=np.float32)
            xloc = np.zeros((TPS * 128, C), dtype=np.float32)
            if cnt:
                kloc[0:cnt] = kA[s:e_]
                xloc[0:cnt] = x[s:e_]
            blk = slice(lj * TPS, (lj + 1) * TPS)
            kx3[:, blk, 0:E] = kloc.reshape(TPS, 128, E).transpose(1, 0, 2)
            kx3[:, blk, E:768] = xloc.reshape(TPS, 128, C).transpose(1, 0, 2)
        kx = to8(kx3.reshape(128, KXT * 768))
        # trig: [128, 2*NP+2*TOPC] = clf | slf | clh+slh(packed)
        tg = np.zeros((128, 2 * NP + 2 * TOPC), dtype=bf)
        tg[:, 0:NP] = trig_c[0:128]
        tg[:, NP : 2 * NP] = trig_s[0:128]
        BOTC = NP - TOPC
        tg[0:64, 2 * NP : 2 * NP + TOPC] = trig_c[128:192, 0:TOPC]
        tg[64:128, 2 * NP : 2 * NP + BOTC] = trig_c[128:192, TOPC:NP]
        tg[0:64, 2 * NP + TOPC : 2 * NP + 2 * TOPC] = trig_s[128:192, 0:TOPC]
        tg[64:128, 2 * NP + TOPC : 2 * NP + TOPC + BOTC] = trig_s[128:192, TOPC:NP]
        im = {"xt": xt, "kx": kx, "trig": tg, "wp": wp, "w8": w8}
        if has_bias:
            im["xb"] = np.ascontiguousarray(bias_col.reshape(NCH, 1, W))
            wb = np.zeros((2, E), dtype=bf)
            wb[0] = (bq[_APERM] * QSC).astype(bf)
            im["wb"] = wb
        in_maps.append(im)

    want_trace = bool(int(os.environ.get("PLATCONV_TRACE", "0")))
    if want_trace:
        want_trace = _ensure_ntff_hook()
    res = run_bass_kernel_spmd(
        nc,
        in_maps,
        core_ids=list(range(NCORES)),
        trace=want_trace,
    )
    last_exec_time_ns = res.exec_time_ns
    last_results = res

    inv = 1.0 / (QSC * VSC)
    out = np.zeros((N, C), dtype=np.float32)
    for d in range(NCORES):
        ot = np.asarray(res.results[d]["outt"], dtype=np.float32)
        for lj in range(GPD):
            gb = d * GPD + lj
            s, e_, cnt = starts[gb], starts[gb + 1], counts[gb]
            if cnt == 0:
                continue
            o = 3 * slot * lj
            for jc in range(3):
                out[s:e_, 128 * jc : 128 * (jc + 1)] = ot[
                    :, o + jc * slot : o + jc * slot + cnt
                ].T
    out *= inv
    out += bo[None, :]
    return out
